# revision 36
# baseline (speedup 1.0000x reference)
"""KMeans assignment kernel (retrieval_knn) for 8 Trainium2 NeuronCores.

Computes argmin_k ||x_n - c_k||^2 for x [262144, 64] f32 against centers
[1024, 64] f32, returning int32 cluster ids [262144].

argmin ||x-c||^2 == argmax s, s = 2x.c - ||c||^2.  Centers are pre-combined
on the host into PAIR sums/differences, so the PE emits, per point, the 512
values sum'_g = (s_2g + s_2g+1)/2 and diff'_g = (s_2g - s_2g+1)/2 (fp16
single-pass matmuls).  The ACT engine computes |diff'| (one Abs per 2-tile
group, psum->SBUF), and ONE custom DVE instruction per group scans BOTH
tiles' 512 pairs (argmax over Src0+Src1 = sum'+|diff'| = max(s_2g, s_2g+1)),
amortizing the per-instruction PSUM access latency.  The stock Spec DSL
cannot reset a MAX scan at page boundaries, so `_lower_batched` extends the
lowering with a SUB_DIM_DONE step state that re-seeds the running-max scan
and the accumulator per page, and routes the accumulator's running value to
the out port; the last element of each page is then the page's argmax
(global Idx, so page j yields j*512 + pair), extracted by a tiny strided
DMA straight to DRAM.  The within-pair winner is resolved exactly on the
host (fp64 compare of the two candidate centers, O(N) numpy).

Hardware walls that shape this design (verified against the NEFF verifier):
a DVE instruction may read at most ONE operand from PSUM, GPSIMD cannot
touch PSUM, ACT cannot combine two tensors, matmul output to PSUM is
fp32-only, and custom DVE ops get no 2x perf modes.  Hence every pair costs
one DVE scan cycle; the remaining lever was the per-instruction overhead,
halved here by the 2-tile batching (DVE ~152.5us busy vs 168.4us unbatched).
"""

import numpy as np

N_POINTS = 262144
N_FEATURES = 64
N_CLUSTERS = 1024
N_PAIRS = N_CLUSTERS // 2               # 512
N_CORES = 8
PTS_PER_CORE = N_POINTS // N_CORES      # 32768
TILE_P = 128                            # points per tile (partition dim)
N_TILES = PTS_PER_CORE // TILE_P        # 256
GR = 2                                  # tiles per batched scan / psum group

_CACHE = {}


def _lower_batched(spec, ver):
    """dve_spec.lower() clone with two extensions: (1) a steady/step FSM
    driven by SUB_DIM_DONE where the step state re-seeds every plain MAX
    scan and the accum stage (per-page reset) while consuming the new
    page's first element; (2) out_sel forced to ALU_OUT so the out stream
    carries the accumulator's running value (running argmax)."""
    import concourse.dve_spec as ds
    from concourse.dve_uop import Trigger

    ds._validate_body(spec, ver)
    spec2 = ds._hoist_stream_invariant_ops(spec)
    scans = ds._collect(spec2.body, ds.Scan)
    latches = ds._collect(spec2.body, ds.Latch)
    placement = ds._build_placement(
        spec2, scans, ds.N_STAGES[ver], ds.N_LANES[ver]
    )
    object.__setattr__(placement, "out_sel", ds.OutSel.ALU_OUT)
    states = ds._build_state_machine(spec2, scans, latches, placement)

    step_ov = {}
    for sc in scans:
        if sc.op == ds.AluOp.MAX and sc._subdim_step is None:
            d = placement.node_stage[sc]
            stg = ds._node_as_stage(sc)          # _Stage(MAX, CURR, expr)
            step_ov[d] = ds._Stage(sc.op, ds.MaxNeg, stg.b)
    if placement.accum_stage is not None:
        step_ov[placement.accum_stage] = ds._Stage(
            spec2.accum, ds.MaxNeg, ds.PREV
        )

    body_lvs = ds._body_scan_leaves(spec2)
    consume = (ds.Src0 in body_lvs, ds.Src1 in body_lvs)
    steady_idx = len(states) - 1
    step_idx = steady_idx + 1
    states[steady_idx] = ds._State(
        placement=placement,
        consume=consume,
        trigger=(Trigger.SRC_TENSOR_DONE, Trigger.SUB_DIM_DONE, Trigger.NONE),
        next=(0, step_idx, 0),
    )
    states.append(
        ds._State(
            placement=placement,
            consume=consume,
            overrides=step_ov,
            trigger=(Trigger.SRC_TENSOR_DONE, Trigger.SUB_DIM_DONE, Trigger.COUNT),
            next=(0, step_idx, steady_idx),
            repeat=1,
        )
    )
    out = [ds._assemble(st) for st in states]
    for u in out:
        u.validate(ver)
    return out


def _register_ops():
    """Register the batched custom DVE op (runtime append to dve_ops.OPS).

    BARGMAX_ANT (row 17): subdim op over [P, S, N]; out stream = running
    argmax of (Src0[k] + Src1[k]) with per-page reset; element (s, N-1) of
    the out stream = page s's argmax with global Idx (so page s yields
    s*N + local index).  uops sha self-pinned at registration."""
    import re
    from dataclasses import dataclass

    from concourse import dve_ops
    from concourse.dve_spec import (
        Spec, Src0, Src1, Idx, MaxNeg, AluOp, scan, eq, select, maxx,
    )
    from concourse.dve_uop import DveOpSpec

    if "BARGMAX_ANT" in dve_ops._SUB_OPCODE_FOR_NAME:
        op = next(op for op in dve_ops.OPS if op.name == "BARGMAX_ANT")
        _CACHE["bargmax_op"] = op
        return op

    def _ref(in0, in1, s0, s1, imm2):
        m = in0.astype(np.float32) + in1
        r = np.maximum.accumulate(m, axis=-1)
        base = (
            np.arange(m.shape[-2], dtype=np.float32)[:, None] * m.shape[-1]
            if m.ndim >= 2
            else 0.0
        )
        idx = np.arange(m.shape[-1], dtype=np.float32) + base
        sel = np.where(m == r, idx, -np.finfo(np.float32).max)
        return np.maximum.accumulate(sel, axis=-1)

    @dataclass(frozen=True)
    class BatchedDveOp(dve_ops.DveOp):
        def compile(self, ver):
            key = (self.name, ver)
            if (r := dve_ops._COMPILE_CACHE.get(key)) is not None:
                return r
            from concourse.dve_spec import _has_src1
            result = DveOpSpec(
                name=self.name,
                opcode=dve_ops.get_dve_sub_opcode(self.name),
                uops=_lower_batched(self.spec, ver),
                rd1_en=_has_src1(self.spec),
            )
            got = result.sha(ver)
            if self.uops_sha.get(ver) != got:
                raise ValueError(f"({ver}: {got} drift")
            dve_ops._COMPILE_CACHE[key] = result
            return result

    _mp = Src0 + Src1
    op = BatchedDveOp(
        "BARGMAX_ANT",
        Spec(
            body=select(eq(_mp, scan(AluOp.MAX, _mp)), Idx, MaxNeg),
            accum=maxx,
            reference=_ref,
        ),
        subdim=True,
        uops_sha={},
    )
    dve_ops.OPS.append(op)
    dve_ops._SUB_OPCODE_FOR_NAME[op.name] = (
        dve_ops._CUSTOM_DVE_ROW_BASE + len(dve_ops.OPS) - 1
    )
    dve_ops.CUSTOM_DVE_SPECS[op.name] = op.spec
    for ver in ("v3", "v4"):
        try:
            op.compile(ver)
        except ValueError as e:
            m = re.search(r"\(%s: ([0-9a-f]+) " % ver, str(e))
            if not m:
                raise
            op.uops_sha[ver] = m.group(1)
            op.compile(ver)
    _CACHE["bargmax_op"] = op
    return op


def _build_bass():
    import concourse.bass as bass
    import concourse.bacc as bacc
    import concourse.mybir as mybir
    import concourse.tile as tile
    from contextlib import ExitStack

    bop = _register_ops()

    f16 = mybir.dt.float16
    f32 = mybir.dt.float32

    nc = bacc.Bacc(None, target_bir_lowering=False)

    xq = nc.declare_dram_parameter("xq", [67, PTS_PER_CORE], f16, isOutput=False)
    ccs = nc.declare_dram_parameter("ccs", [67, N_PAIRS], f16, isOutput=False)
    ccd = nc.declare_dram_parameter("ccd", [67, N_PAIRS], f16, isOutput=False)
    out = nc.declare_dram_parameter("out", [128, N_TILES], f16, isOutput=True)

    with tile.TileContext(nc) as tc, ExitStack() as ctx:
        const_pool = ctx.enter_context(tc.tile_pool(name="const", bufs=1))
        psum_pool = ctx.enter_context(
            tc.tile_pool(name="psum", bufs=2, space=bass.MemorySpace.PSUM)
        )
        abs_pool = ctx.enter_context(tc.tile_pool(name="absd", bufs=3))
        scr_pool = ctx.enter_context(tc.tile_pool(name="scr", bufs=2))
        idx_pool = ctx.enter_context(tc.tile_pool(name="idx", bufs=3))
        SB = 16                               # groups per scratch super-batch

        # dummy 1-elem Abs absorbs the 1283ns ACT table load off the critical
        # path; its memset goes FIRST on the gpsimd queue
        xq_t = const_pool.tile([67, PTS_PER_CORE], f16)
        ccs_t = const_pool.tile([67, N_PAIRS], f16)
        ccd_t = const_pool.tile([67, N_PAIRS], f16)
        dummy_in = const_pool.tile([128, 1], f32)
        nc.gpsimd.memset(dummy_in[:], 0)
        dummy_out = const_pool.tile([128, 1], f32)
        nc.scalar.activation(
            dummy_out[:], dummy_in[:], mybir.ActivationFunctionType.Abs
        )
        # a tiny matmul on the memset tile pins pe_busy_start at ~100ns so
        # even the first real matmuls run at the max p-state (the ramp clock
        # never resets on idle)
        warm = psum_pool.tile([128, GR, N_PAIRS], f32, tag="sums")
        nc.tensor.matmul(
            warm[0:1, 0, 0:1], dummy_in[:], dummy_in[:],
            start=True, stop=True,
        )
        # constants split across both cheap queues (dif path gates the head)
        nc.gpsimd.dma_start(ccd_t[:, 0:256], ccd[:, 0:256])
        nc.gpsimd.dma_start(ccs_t[:], ccs[:])
        nc.sync.dma_start(xq_t[:, 0 : 2 * TILE_P], xq[:, 0 : 2 * TILE_P])
        nc.sync.dma_start(ccd_t[:, 256:512], ccd[:, 256:512])
        # rest of x: small chunks first, then 1024-pt chunks, both queues
        CH_EDGES = [2 * TILE_P, 512, 1024]
        while CH_EDGES[-1] < PTS_PER_CORE:
            CH_EDGES.append(min(CH_EDGES[-1] + 1024, PTS_PER_CORE))
        for ci in range(len(CH_EDGES) - 1):
            csl = slice(CH_EDGES[ci], CH_EDGES[ci + 1])
            q = nc.sync if ci % 2 else nc.gpsimd
            q.dma_start(xq_t[:, csl], xq[:, csl])

        # groups: single-tile first and last (shorter head chain and drain),
        # 2-tile batched in between; super-batches of ~32 tiles for the
        # scratch/extraction machinery
        groups = (
            [(0, 1), (1, 1)]
            + [(2 + 2 * k, 2) for k in range(126)]
            + [(254, 1), (255, 1)]
        )
        sb_sizes = [32] * 7 + [28, 4]
        sb_of, acc = [], 0
        for sbi, sz in enumerate(sb_sizes):
            n = 0
            while n < sz:
                n += groups[len(sb_of)][1]
                sb_of.append(sbi)
            acc += sz
        tpos = 0
        gidx = 0
        for gi, (t0g, gr) in enumerate(groups):
            if tpos == 0:
                sbt = sb_sizes[sb_of[gi]]
                sb_base = t0g
                scrbig = scr_pool.tile([128, 33, N_PAIRS], f16)
                accb = idx_pool.tile([128, 17], f32)
                gidx = 0
            # separate psum pools so difs free after the Abs and sums after
            # the scan -- 2 bufs each gives a 3-stage PE->ACT->DVE pipeline
            pss = psum_pool.tile([128, GR, N_PAIRS], f32, tag="sums")
            psd = psum_pool.tile([128, GR, N_PAIRS], f32, tag="difs")
            for j in range(gr):           # difs first: they gate the Abs
                t = t0g + j
                tsl = slice(t * TILE_P, (t + 1) * TILE_P)
                nc.tensor.matmul(
                    psd[:, j, :], xq_t[:, tsl], ccd_t[:],
                    start=True, stop=True,
                )
            for j in range(gr):
                t = t0g + j
                tsl = slice(t * TILE_P, (t + 1) * TILE_P)
                nc.tensor.matmul(
                    pss[:, j, :], xq_t[:, tsl], ccs_t[:],
                    start=True, stop=True,
                )
            absd = abs_pool.tile([128, GR, N_PAIRS], f32)
            nc.scalar.activation(
                absd[:, 0:gr, :], psd[:, 0:gr, :],
                mybir.ActivationFunctionType.Abs,
            )
            nc.vector._custom_dve(
                bop,
                out=scrbig[:, tpos : tpos + gr, :],
                in0=pss[:, 0:gr, :],
                in1=absd[:, 0:gr, :],
                accum_out=accb[:, gidx : gidx + 1],
            )
            tpos += gr
            gidx += 1
            if tpos == sbt:
                # page argmaxes = last element of each page: one cheap DVE
                # copy gathers them, then one contiguous DMA
                idxb = idx_pool.tile([128, 33, 1], f16)
                nc.vector.tensor_copy(
                    idxb[:, 0:sbt, :],
                    scrbig[:, 0:sbt, N_PAIRS - 1 : N_PAIRS],
                )
                nc.sync.dma_start(
                    out[:, sb_base : sb_base + sbt], idxb[:, 0:sbt, :]
                )
                tpos = 0

    nc.compile()
    return nc


def _casc3(A):
    """3-row fp16 cascade summing (exactly, up to fp16 subnormal flush) to A."""
    f16 = np.float16
    n1 = A.astype(f16)
    r1 = A - n1.astype(np.float64)
    n2 = r1.astype(f16)
    n3 = (r1 - n2.astype(np.float64)).astype(f16)
    return n1, n2, n3


def _prep(x: np.ndarray, centers: np.ndarray):
    f16 = np.float16
    xd = x.astype(np.float64)
    cd = centers.astype(np.float64)

    xq = np.empty((67, N_POINTS), f16)
    xq[0:64] = np.ascontiguousarray(xd.T).astype(f16)
    xq[64:67] = f16(1.0)

    cn = (cd * cd).sum(1)
    csum = cd[0::2] + cd[1::2]                  # [512, 64]
    cdif = cd[0::2] - cd[1::2]
    cnsum = (cn[0::2] + cn[1::2]) / 2.0
    cndif = (cn[0::2] - cn[1::2]) / 2.0

    # device computes sum'_g = x.csum - cnsum = (s_2g + s_2g+1)/2
    #             and diff'_g = x.cdif - cndif = (s_2g - s_2g+1)/2
    ccs = np.empty((67, N_PAIRS), f16)
    ccs[0:64] = csum.T.astype(f16)
    ccs[64], ccs[65], ccs[66] = _casc3(-cnsum)
    ccd = np.empty((67, N_PAIRS), f16)
    ccd[0:64] = cdif.T.astype(f16)
    ccd[64], ccd[65], ccd[66] = _casc3(-cndif)
    return xq, ccs, ccd


def kernel(x: np.ndarray, centers: np.ndarray) -> np.ndarray:
    import sys
    if "/opt/trn_rl_repo" not in sys.path:
        sys.path.insert(0, "/opt/trn_rl_repo")
    from concourse.bass_utils import run_bass_kernel_spmd

    x = np.asarray(x, dtype=np.float32)
    centers = np.asarray(centers, dtype=np.float32)

    xq, ccs, ccd = _prep(x, centers)

    if "nc" not in _CACHE:
        _CACHE["nc"] = _build_bass()
    nc = _CACHE["nc"]

    in_maps = []
    for c in range(N_CORES):
        sl = slice(c * PTS_PER_CORE, (c + 1) * PTS_PER_CORE)
        in_maps.append(
            {
                "xq": np.ascontiguousarray(xq[:, sl]),
                "ccs": ccs,
                "ccd": ccd,
            }
        )

    res = run_bass_kernel_spmd(nc, in_maps, list(range(N_CORES)))

    outs = []
    for c in range(N_CORES):
        o = res.results[c]["out"]                 # [128, N_TILES] f16 global idx
        a = np.asarray(o).astype(np.int64)        # page j value = j*512 + pair
        outs.append((a % N_PAIRS).T.reshape(-1))  # point t*128+p -> pair
    g = np.concatenate(outs)                      # winning pair per point

    # within-pair refinement on host: exact fp64 distance compare of the two
    # candidate centers; ties pick the first (matches reference argmin)
    xd = x.astype(np.float64)
    cd = centers.astype(np.float64)
    c0 = cd[2 * g]
    c1 = cd[2 * g + 1]
    d0 = ((xd - c0) ** 2).sum(1)
    d1 = ((xd - c1) ** 2).sum(1)
    ids = np.where(d1 < d0, 2 * g + 1, 2 * g)
    return ids.astype(np.int32)


if __name__ == "__main__":
    rng = np.random.default_rng(0)
    x = rng.normal(size=(N_POINTS, N_FEATURES)).astype(np.float32)
    c = rng.normal(size=(N_CLUSTERS, N_FEATURES)).astype(np.float32)
    ids = kernel(x=x, centers=c)
    d = (
        np.sum(x * x, 1)[:, None]
        - 2.0 * (x @ c.T)
        + np.sum(c * c, 1)[None, :]
    )
    ref = np.argmin(np.abs(d), axis=1)
    print("mismatch:", np.mean(ids != ref))


# revision 37
# speedup vs baseline: 1.0008x; 1.0008x over previous
"""KMeans assignment kernel (retrieval_knn) for 8 Trainium2 NeuronCores.

Computes argmin_k ||x_n - c_k||^2 for x [262144, 64] f32 against centers
[1024, 64] f32, returning int32 cluster ids [262144].

argmin ||x-c||^2 == argmax s, s = 2x.c - ||c||^2.  Centers are pre-combined
on the host into PAIR sums/differences, so the PE emits, per point, the 512
values sum'_g = (s_2g + s_2g+1)/2 and diff'_g = (s_2g - s_2g+1)/2 (fp16
single-pass matmuls).  The ACT engine computes |diff'| (one Abs per 2-tile
group, psum->SBUF), and ONE custom DVE instruction per group scans BOTH
tiles' 512 pairs (argmax over Src0+Src1 = sum'+|diff'| = max(s_2g, s_2g+1)),
amortizing the per-instruction PSUM access latency.  The stock Spec DSL
cannot reset a MAX scan at page boundaries, so `_lower_batched` extends the
lowering with a SUB_DIM_DONE step state that re-seeds the running-max scan
and the accumulator per page, and routes the accumulator's running value to
the out port; the last element of each page is then the page's argmax
(global Idx, so page j yields j*512 + pair), extracted by a tiny strided
DMA straight to DRAM.  The within-pair winner is resolved exactly on the
host (fp64 compare of the two candidate centers, O(N) numpy).

Hardware walls that shape this design (verified against the NEFF verifier):
a DVE instruction may read at most ONE operand from PSUM, GPSIMD cannot
touch PSUM, ACT cannot combine two tensors, matmul output to PSUM is
fp32-only, and custom DVE ops get no 2x perf modes.  Hence every pair costs
one DVE scan cycle; the remaining lever was the per-instruction overhead,
halved here by the 2-tile batching (DVE ~152.5us busy vs 168.4us unbatched).
"""

import numpy as np

N_POINTS = 262144
N_FEATURES = 64
N_CLUSTERS = 1024
N_PAIRS = N_CLUSTERS // 2               # 512
N_CORES = 8
PTS_PER_CORE = N_POINTS // N_CORES      # 32768
TILE_P = 128                            # points per tile (partition dim)
N_TILES = PTS_PER_CORE // TILE_P        # 256
GR = 2                                  # tiles per batched scan / psum group

_CACHE = {}


def _lower_batched(spec, ver):
    """dve_spec.lower() clone with two extensions: (1) a steady/step FSM
    driven by SUB_DIM_DONE where the step state re-seeds every plain MAX
    scan and the accum stage (per-page reset) while consuming the new
    page's first element; (2) out_sel forced to ALU_OUT so the out stream
    carries the accumulator's running value (running argmax)."""
    import concourse.dve_spec as ds
    from concourse.dve_uop import Trigger

    ds._validate_body(spec, ver)
    spec2 = ds._hoist_stream_invariant_ops(spec)
    scans = ds._collect(spec2.body, ds.Scan)
    latches = ds._collect(spec2.body, ds.Latch)
    placement = ds._build_placement(
        spec2, scans, ds.N_STAGES[ver], ds.N_LANES[ver]
    )
    object.__setattr__(placement, "out_sel", ds.OutSel.ALU_OUT)
    states = ds._build_state_machine(spec2, scans, latches, placement)

    step_ov = {}
    for sc in scans:
        if sc.op == ds.AluOp.MAX and sc._subdim_step is None:
            d = placement.node_stage[sc]
            stg = ds._node_as_stage(sc)          # _Stage(MAX, CURR, expr)
            step_ov[d] = ds._Stage(sc.op, ds.MaxNeg, stg.b)
    if placement.accum_stage is not None:
        step_ov[placement.accum_stage] = ds._Stage(
            spec2.accum, ds.MaxNeg, ds.PREV
        )

    body_lvs = ds._body_scan_leaves(spec2)
    consume = (ds.Src0 in body_lvs, ds.Src1 in body_lvs)
    steady_idx = len(states) - 1
    step_idx = steady_idx + 1
    states[steady_idx] = ds._State(
        placement=placement,
        consume=consume,
        trigger=(Trigger.SRC_TENSOR_DONE, Trigger.SUB_DIM_DONE, Trigger.NONE),
        next=(0, step_idx, 0),
    )
    states.append(
        ds._State(
            placement=placement,
            consume=consume,
            overrides=step_ov,
            trigger=(Trigger.SRC_TENSOR_DONE, Trigger.SUB_DIM_DONE, Trigger.COUNT),
            next=(0, step_idx, steady_idx),
            repeat=1,
        )
    )
    out = [ds._assemble(st) for st in states]
    for u in out:
        u.validate(ver)
    return out


def _register_ops():
    """Register the batched custom DVE op (runtime append to dve_ops.OPS).

    BARGMAX_ANT (row 17): subdim op over [P, S, N]; out stream = running
    argmax of (Src0[k] + Src1[k]) with per-page reset; element (s, N-1) of
    the out stream = page s's argmax with global Idx (so page s yields
    s*N + local index).  uops sha self-pinned at registration."""
    import re
    from dataclasses import dataclass

    from concourse import dve_ops
    from concourse.dve_spec import (
        Spec, Src0, Src1, Idx, MaxNeg, AluOp, scan, eq, select, maxx,
    )
    from concourse.dve_uop import DveOpSpec

    if "BARGMAX_ANT" in dve_ops._SUB_OPCODE_FOR_NAME:
        op = next(op for op in dve_ops.OPS if op.name == "BARGMAX_ANT")
        _CACHE["bargmax_op"] = op
        return op

    def _ref(in0, in1, s0, s1, imm2):
        m = in0.astype(np.float32) + in1
        r = np.maximum.accumulate(m, axis=-1)
        base = (
            np.arange(m.shape[-2], dtype=np.float32)[:, None] * m.shape[-1]
            if m.ndim >= 2
            else 0.0
        )
        idx = np.arange(m.shape[-1], dtype=np.float32) + base
        sel = np.where(m == r, idx, -np.finfo(np.float32).max)
        return np.maximum.accumulate(sel, axis=-1)

    @dataclass(frozen=True)
    class BatchedDveOp(dve_ops.DveOp):
        def compile(self, ver):
            key = (self.name, ver)
            if (r := dve_ops._COMPILE_CACHE.get(key)) is not None:
                return r
            from concourse.dve_spec import _has_src1
            result = DveOpSpec(
                name=self.name,
                opcode=dve_ops.get_dve_sub_opcode(self.name),
                uops=_lower_batched(self.spec, ver),
                rd1_en=_has_src1(self.spec),
            )
            got = result.sha(ver)
            if self.uops_sha.get(ver) != got:
                raise ValueError(f"({ver}: {got} drift")
            dve_ops._COMPILE_CACHE[key] = result
            return result

    _mp = Src0 + Src1
    op = BatchedDveOp(
        "BARGMAX_ANT",
        Spec(
            body=select(eq(_mp, scan(AluOp.MAX, _mp)), Idx, MaxNeg),
            accum=maxx,
            reference=_ref,
        ),
        subdim=True,
        uops_sha={},
    )
    dve_ops.OPS.append(op)
    dve_ops._SUB_OPCODE_FOR_NAME[op.name] = (
        dve_ops._CUSTOM_DVE_ROW_BASE + len(dve_ops.OPS) - 1
    )
    dve_ops.CUSTOM_DVE_SPECS[op.name] = op.spec
    for ver in ("v3", "v4"):
        try:
            op.compile(ver)
        except ValueError as e:
            m = re.search(r"\(%s: ([0-9a-f]+) " % ver, str(e))
            if not m:
                raise
            op.uops_sha[ver] = m.group(1)
            op.compile(ver)
    _CACHE["bargmax_op"] = op
    return op


def _build_bass():
    import concourse.bass as bass
    import concourse.bacc as bacc
    import concourse.mybir as mybir
    import concourse.tile as tile
    from contextlib import ExitStack

    bop = _register_ops()

    f16 = mybir.dt.float16
    f32 = mybir.dt.float32

    nc = bacc.Bacc(None, target_bir_lowering=False)

    xq = nc.declare_dram_parameter("xq", [67, PTS_PER_CORE], f16, isOutput=False)
    ccs = nc.declare_dram_parameter("ccs", [67, N_PAIRS], f16, isOutput=False)
    ccd = nc.declare_dram_parameter("ccd", [67, N_PAIRS], f16, isOutput=False)
    out = nc.declare_dram_parameter("out", [128, N_TILES], f16, isOutput=True)

    with tile.TileContext(nc) as tc, ExitStack() as ctx:
        const_pool = ctx.enter_context(tc.tile_pool(name="const", bufs=1))
        psum_pool = ctx.enter_context(
            tc.tile_pool(name="psum", bufs=2, space=bass.MemorySpace.PSUM)
        )
        abs_pool = ctx.enter_context(tc.tile_pool(name="absd", bufs=3))
        scr_pool = ctx.enter_context(tc.tile_pool(name="scr", bufs=2))
        idx_pool = ctx.enter_context(tc.tile_pool(name="idx", bufs=3))
        SB = 16                               # groups per scratch super-batch

        # dummy 1-elem Abs absorbs the 1283ns ACT table load off the critical
        # path; its memset goes FIRST on the gpsimd queue
        xq_t = const_pool.tile([67, PTS_PER_CORE], f16)
        ccs_t = const_pool.tile([67, N_PAIRS], f16)
        ccd_t = const_pool.tile([67, N_PAIRS], f16)
        dummy_in = const_pool.tile([128, 1], f32)
        nc.gpsimd.memset(dummy_in[:], 0)
        dummy_out = const_pool.tile([128, 1], f32)
        nc.scalar.activation(
            dummy_out[:], dummy_in[:], mybir.ActivationFunctionType.Abs
        )
        # a tiny matmul on the memset tile pins pe_busy_start at ~100ns so
        # even the first real matmuls run at the max p-state (the ramp clock
        # never resets on idle)
        warm = psum_pool.tile([128, GR, N_PAIRS], f32, tag="sums")
        nc.tensor.matmul(
            warm[0:1, 0, 0:1], dummy_in[:], dummy_in[:],
            start=True, stop=True,
        )
        # constants split across both cheap queues (dif path gates the head)
        nc.gpsimd.dma_start(ccd_t[:, 0:256], ccd[:, 0:256])
        nc.gpsimd.dma_start(ccs_t[:], ccs[:])
        nc.sync.dma_start(xq_t[:, 0 : 2 * TILE_P], xq[:, 0 : 2 * TILE_P])
        nc.sync.dma_start(ccd_t[:, 256:512], ccd[:, 256:512])
        # rest of x: small chunks first, then 1024-pt chunks, both queues
        CH_EDGES = [2 * TILE_P, 512, 1024]
        while CH_EDGES[-1] < PTS_PER_CORE:
            CH_EDGES.append(min(CH_EDGES[-1] + 1024, PTS_PER_CORE))
        for ci in range(len(CH_EDGES) - 1):
            csl = slice(CH_EDGES[ci], CH_EDGES[ci + 1])
            q = nc.sync if ci % 2 else nc.gpsimd
            q.dma_start(xq_t[:, csl], xq[:, csl])

        # groups: single-tile first and last (shorter head chain and drain),
        # 2-tile batched in between; super-batches of ~32 tiles for the
        # scratch/extraction machinery
        groups = [(0, 1)] + [(1 + 2 * k, 2) for k in range(127)] + [(255, 1)]
        sb_sizes = [31] + [32] * 6 + [33]
        sb_of, acc = [], 0
        for sbi, sz in enumerate(sb_sizes):
            n = 0
            while n < sz:
                n += groups[len(sb_of)][1]
                sb_of.append(sbi)
            acc += sz
        tpos = 0
        gidx = 0
        for gi, (t0g, gr) in enumerate(groups):
            if tpos == 0:
                sbt = sb_sizes[sb_of[gi]]
                sb_base = t0g
                scrbig = scr_pool.tile([128, 33, N_PAIRS], f16)
                accb = idx_pool.tile([128, 17], f32)
                gidx = 0
            # separate psum pools so difs free after the Abs and sums after
            # the scan -- 2 bufs each gives a 3-stage PE->ACT->DVE pipeline
            pss = psum_pool.tile([128, GR, N_PAIRS], f32, tag="sums")
            psd = psum_pool.tile([128, GR, N_PAIRS], f32, tag="difs")
            for j in range(gr):           # difs first: they gate the Abs
                t = t0g + j
                tsl = slice(t * TILE_P, (t + 1) * TILE_P)
                nc.tensor.matmul(
                    psd[:, j, :], xq_t[:, tsl], ccd_t[:],
                    start=True, stop=True,
                )
            for j in range(gr):
                t = t0g + j
                tsl = slice(t * TILE_P, (t + 1) * TILE_P)
                nc.tensor.matmul(
                    pss[:, j, :], xq_t[:, tsl], ccs_t[:],
                    start=True, stop=True,
                )
            absd = abs_pool.tile([128, GR, N_PAIRS], f32)
            nc.scalar.activation(
                absd[:, 0:gr, :], psd[:, 0:gr, :],
                mybir.ActivationFunctionType.Abs,
            )
            nc.vector._custom_dve(
                bop,
                out=scrbig[:, tpos : tpos + gr, :],
                in0=pss[:, 0:gr, :],
                in1=absd[:, 0:gr, :],
                accum_out=accb[:, gidx : gidx + 1],
            )
            tpos += gr
            gidx += 1
            if tpos == sbt:
                # page argmaxes = last element of each page: one cheap DVE
                # copy gathers them, then one contiguous DMA
                idxb = idx_pool.tile([128, 33, 1], f16)
                nc.vector.tensor_copy(
                    idxb[:, 0:sbt, :],
                    scrbig[:, 0:sbt, N_PAIRS - 1 : N_PAIRS],
                )
                nc.sync.dma_start(
                    out[:, sb_base : sb_base + sbt], idxb[:, 0:sbt, :]
                )
                tpos = 0

    nc.compile()
    return nc


def _casc3(A):
    """3-row fp16 cascade summing (exactly, up to fp16 subnormal flush) to A."""
    f16 = np.float16
    n1 = A.astype(f16)
    r1 = A - n1.astype(np.float64)
    n2 = r1.astype(f16)
    n3 = (r1 - n2.astype(np.float64)).astype(f16)
    return n1, n2, n3


def _prep(x: np.ndarray, centers: np.ndarray):
    f16 = np.float16
    xd = x.astype(np.float64)
    cd = centers.astype(np.float64)

    xq = np.empty((67, N_POINTS), f16)
    xq[0:64] = np.ascontiguousarray(xd.T).astype(f16)
    xq[64:67] = f16(1.0)

    cn = (cd * cd).sum(1)
    csum = cd[0::2] + cd[1::2]                  # [512, 64]
    cdif = cd[0::2] - cd[1::2]
    cnsum = (cn[0::2] + cn[1::2]) / 2.0
    cndif = (cn[0::2] - cn[1::2]) / 2.0

    # device computes sum'_g = x.csum - cnsum = (s_2g + s_2g+1)/2
    #             and diff'_g = x.cdif - cndif = (s_2g - s_2g+1)/2
    ccs = np.empty((67, N_PAIRS), f16)
    ccs[0:64] = csum.T.astype(f16)
    ccs[64], ccs[65], ccs[66] = _casc3(-cnsum)
    ccd = np.empty((67, N_PAIRS), f16)
    ccd[0:64] = cdif.T.astype(f16)
    ccd[64], ccd[65], ccd[66] = _casc3(-cndif)
    return xq, ccs, ccd


def kernel(x: np.ndarray, centers: np.ndarray) -> np.ndarray:
    import sys
    if "/opt/trn_rl_repo" not in sys.path:
        sys.path.insert(0, "/opt/trn_rl_repo")
    from concourse.bass_utils import run_bass_kernel_spmd

    x = np.asarray(x, dtype=np.float32)
    centers = np.asarray(centers, dtype=np.float32)

    xq, ccs, ccd = _prep(x, centers)

    if "nc" not in _CACHE:
        _CACHE["nc"] = _build_bass()
    nc = _CACHE["nc"]

    in_maps = []
    for c in range(N_CORES):
        sl = slice(c * PTS_PER_CORE, (c + 1) * PTS_PER_CORE)
        in_maps.append(
            {
                "xq": np.ascontiguousarray(xq[:, sl]),
                "ccs": ccs,
                "ccd": ccd,
            }
        )

    res = run_bass_kernel_spmd(nc, in_maps, list(range(N_CORES)))

    outs = []
    for c in range(N_CORES):
        o = res.results[c]["out"]                 # [128, N_TILES] f16 global idx
        a = np.asarray(o).astype(np.int64)        # page j value = j*512 + pair
        outs.append((a % N_PAIRS).T.reshape(-1))  # point t*128+p -> pair
    g = np.concatenate(outs)                      # winning pair per point

    # within-pair refinement on host: exact fp64 distance compare of the two
    # candidate centers; ties pick the first (matches reference argmin)
    xd = x.astype(np.float64)
    cd = centers.astype(np.float64)
    c0 = cd[2 * g]
    c1 = cd[2 * g + 1]
    d0 = ((xd - c0) ** 2).sum(1)
    d1 = ((xd - c1) ** 2).sum(1)
    ids = np.where(d1 < d0, 2 * g + 1, 2 * g)
    return ids.astype(np.int32)


if __name__ == "__main__":
    rng = np.random.default_rng(0)
    x = rng.normal(size=(N_POINTS, N_FEATURES)).astype(np.float32)
    c = rng.normal(size=(N_CLUSTERS, N_FEATURES)).astype(np.float32)
    ids = kernel(x=x, centers=c)
    d = (
        np.sum(x * x, 1)[:, None]
        - 2.0 * (x @ c.T)
        + np.sum(c * c, 1)[None, :]
    )
    ref = np.argmin(np.abs(d), axis=1)
    print("mismatch:", np.mean(ids != ref))


# revision 38
# speedup vs baseline: 1.0045x; 1.0037x over previous
"""KMeans assignment kernel (retrieval_knn) for 8 Trainium2 NeuronCores.

Computes argmin_k ||x_n - c_k||^2 for x [262144, 64] f32 against centers
[1024, 64] f32, returning int32 cluster ids [262144].

argmin ||x-c||^2 == argmax s, s = 2x.c - ||c||^2.  Centers are pre-combined
on the host into PAIR sums/differences, so the PE emits, per point, the 512
values sum'_g = (s_2g + s_2g+1)/2 and diff'_g = (s_2g - s_2g+1)/2 (fp16
single-pass matmuls).  The ACT engine computes |diff'| (one Abs per 2-tile
group, psum->SBUF), and ONE custom DVE instruction per group scans BOTH
tiles' 512 pairs (argmax over Src0+Src1 = sum'+|diff'| = max(s_2g, s_2g+1)),
amortizing the per-instruction PSUM access latency.  The stock Spec DSL
cannot reset a MAX scan at page boundaries, so `_lower_batched` extends the
lowering with a SUB_DIM_DONE step state that re-seeds the running-max scan
and the accumulator per page, and routes the accumulator's running value to
the out port; the last element of each page is then the page's argmax
(global Idx, so page j yields j*512 + pair), extracted by a tiny strided
DMA straight to DRAM.  The within-pair winner is resolved exactly on the
host (fp64 compare of the two candidate centers, O(N) numpy).

Hardware walls that shape this design (verified against the NEFF verifier):
a DVE instruction may read at most ONE operand from PSUM, GPSIMD cannot
touch PSUM, ACT cannot combine two tensors, matmul output to PSUM is
fp32-only, and custom DVE ops get no 2x perf modes.  Hence every pair costs
one DVE scan cycle; the remaining lever was the per-instruction overhead,
halved here by the 2-tile batching (DVE ~152.5us busy vs 168.4us unbatched).
"""

import numpy as np

N_POINTS = 262144
N_FEATURES = 64
N_CLUSTERS = 1024
N_PAIRS = N_CLUSTERS // 2               # 512
N_CORES = 8
PTS_PER_CORE = N_POINTS // N_CORES      # 32768
TILE_P = 128                            # points per tile (partition dim)
N_TILES = PTS_PER_CORE // TILE_P        # 256
GR = 2                                  # tiles per batched scan / psum group

_CACHE = {}


def _lower_batched(spec, ver):
    """dve_spec.lower() clone with two extensions: (1) a steady/step FSM
    driven by SUB_DIM_DONE where the step state re-seeds every plain MAX
    scan and the accum stage (per-page reset) while consuming the new
    page's first element; (2) out_sel forced to ALU_OUT so the out stream
    carries the accumulator's running value (running argmax)."""
    import concourse.dve_spec as ds
    from concourse.dve_uop import Trigger

    ds._validate_body(spec, ver)
    spec2 = ds._hoist_stream_invariant_ops(spec)
    scans = ds._collect(spec2.body, ds.Scan)
    latches = ds._collect(spec2.body, ds.Latch)
    placement = ds._build_placement(
        spec2, scans, ds.N_STAGES[ver], ds.N_LANES[ver]
    )
    object.__setattr__(placement, "out_sel", ds.OutSel.ALU_OUT)
    states = ds._build_state_machine(spec2, scans, latches, placement)

    step_ov = {}
    for sc in scans:
        if sc.op == ds.AluOp.MAX and sc._subdim_step is None:
            d = placement.node_stage[sc]
            stg = ds._node_as_stage(sc)          # _Stage(MAX, CURR, expr)
            step_ov[d] = ds._Stage(sc.op, ds.MaxNeg, stg.b)
    if placement.accum_stage is not None:
        step_ov[placement.accum_stage] = ds._Stage(
            spec2.accum, ds.MaxNeg, ds.PREV
        )

    body_lvs = ds._body_scan_leaves(spec2)
    consume = (ds.Src0 in body_lvs, ds.Src1 in body_lvs)
    steady_idx = len(states) - 1
    step_idx = steady_idx + 1
    states[steady_idx] = ds._State(
        placement=placement,
        consume=consume,
        trigger=(Trigger.SRC_TENSOR_DONE, Trigger.SUB_DIM_DONE, Trigger.NONE),
        next=(0, step_idx, 0),
    )
    states.append(
        ds._State(
            placement=placement,
            consume=consume,
            overrides=step_ov,
            trigger=(Trigger.SRC_TENSOR_DONE, Trigger.SUB_DIM_DONE, Trigger.COUNT),
            next=(0, step_idx, steady_idx),
            repeat=1,
        )
    )
    out = [ds._assemble(st) for st in states]
    for u in out:
        u.validate(ver)
    return out


def _register_ops():
    """Register the batched custom DVE op (runtime append to dve_ops.OPS).

    BARGMAX_ANT (row 17): subdim op over [P, S, N]; out stream = running
    argmax of (Src0[k] + Src1[k]) with per-page reset; element (s, N-1) of
    the out stream = page s's argmax with global Idx (so page s yields
    s*N + local index).  uops sha self-pinned at registration."""
    import re
    from dataclasses import dataclass

    from concourse import dve_ops
    from concourse.dve_spec import (
        Spec, Src0, Src1, Idx, MaxNeg, AluOp, scan, eq, select, maxx,
    )
    from concourse.dve_uop import DveOpSpec

    if "BARGMAX_ANT" in dve_ops._SUB_OPCODE_FOR_NAME:
        op = next(op for op in dve_ops.OPS if op.name == "BARGMAX_ANT")
        _CACHE["bargmax_op"] = op
        return op

    def _ref(in0, in1, s0, s1, imm2):
        m = in0.astype(np.float32) + in1
        r = np.maximum.accumulate(m, axis=-1)
        base = (
            np.arange(m.shape[-2], dtype=np.float32)[:, None] * m.shape[-1]
            if m.ndim >= 2
            else 0.0
        )
        idx = np.arange(m.shape[-1], dtype=np.float32) + base
        sel = np.where(m == r, idx, -np.finfo(np.float32).max)
        return np.maximum.accumulate(sel, axis=-1)

    @dataclass(frozen=True)
    class BatchedDveOp(dve_ops.DveOp):
        def compile(self, ver):
            key = (self.name, ver)
            if (r := dve_ops._COMPILE_CACHE.get(key)) is not None:
                return r
            from concourse.dve_spec import _has_src1
            result = DveOpSpec(
                name=self.name,
                opcode=dve_ops.get_dve_sub_opcode(self.name),
                uops=_lower_batched(self.spec, ver),
                rd1_en=_has_src1(self.spec),
            )
            got = result.sha(ver)
            if self.uops_sha.get(ver) != got:
                raise ValueError(f"({ver}: {got} drift")
            dve_ops._COMPILE_CACHE[key] = result
            return result

    _mp = Src0 + Src1
    op = BatchedDveOp(
        "BARGMAX_ANT",
        Spec(
            body=select(eq(_mp, scan(AluOp.MAX, _mp)), Idx, MaxNeg),
            accum=maxx,
            reference=_ref,
        ),
        subdim=True,
        uops_sha={},
    )
    dve_ops.OPS.append(op)
    dve_ops._SUB_OPCODE_FOR_NAME[op.name] = (
        dve_ops._CUSTOM_DVE_ROW_BASE + len(dve_ops.OPS) - 1
    )
    dve_ops.CUSTOM_DVE_SPECS[op.name] = op.spec
    for ver in ("v3", "v4"):
        try:
            op.compile(ver)
        except ValueError as e:
            m = re.search(r"\(%s: ([0-9a-f]+) " % ver, str(e))
            if not m:
                raise
            op.uops_sha[ver] = m.group(1)
            op.compile(ver)
    _CACHE["bargmax_op"] = op
    return op


def _build_bass():
    import concourse.bass as bass
    import concourse.bacc as bacc
    import concourse.mybir as mybir
    import concourse.tile as tile
    from contextlib import ExitStack

    bop = _register_ops()

    f16 = mybir.dt.float16
    f32 = mybir.dt.float32

    nc = bacc.Bacc(None, target_bir_lowering=False)

    xq = nc.declare_dram_parameter("xq", [67, PTS_PER_CORE], f16, isOutput=False)
    ccs = nc.declare_dram_parameter("ccs", [67, N_PAIRS], f16, isOutput=False)
    ccd = nc.declare_dram_parameter("ccd", [67, N_PAIRS], f16, isOutput=False)
    out = nc.declare_dram_parameter("out", [128, N_TILES], f16, isOutput=True)

    with tile.TileContext(nc) as tc, ExitStack() as ctx:
        const_pool = ctx.enter_context(tc.tile_pool(name="const", bufs=1))
        psum_pool = ctx.enter_context(
            tc.tile_pool(name="psum", bufs=2, space=bass.MemorySpace.PSUM)
        )
        abs_pool = ctx.enter_context(tc.tile_pool(name="absd", bufs=3))
        scr_pool = ctx.enter_context(tc.tile_pool(name="scr", bufs=2))
        idx_pool = ctx.enter_context(tc.tile_pool(name="idx", bufs=3))
        SB = 16                               # groups per scratch super-batch

        # dummy 1-elem Abs absorbs the 1283ns ACT table load off the critical
        # path; its memset goes FIRST on the gpsimd queue
        xq_t = const_pool.tile([67, PTS_PER_CORE], f16)
        ccs_t = const_pool.tile([67, N_PAIRS], f16)
        ccd_t = const_pool.tile([67, N_PAIRS], f16)
        dummy_in = const_pool.tile([128, 1], f32)
        nc.gpsimd.memset(dummy_in[:], 0)
        dummy_out = const_pool.tile([128, 1], f32)
        nc.scalar.activation(
            dummy_out[:], dummy_in[:], mybir.ActivationFunctionType.Abs
        )
        # a tiny matmul on the memset tile pins pe_busy_start at ~100ns so
        # even the first real matmuls run at the max p-state (the ramp clock
        # never resets on idle)
        warm = psum_pool.tile([128, GR, N_PAIRS], f32, tag="sums")
        nc.tensor.matmul(
            warm[0:1, 0, 0:1], dummy_in[:], dummy_in[:],
            start=True, stop=True,
        )
        # constants split across both cheap queues (dif path gates the head)
        nc.gpsimd.dma_start(ccd_t[:, 0:256], ccd[:, 0:256])
        nc.gpsimd.dma_start(ccs_t[:], ccs[:])
        nc.sync.dma_start(xq_t[:, 0 : 2 * TILE_P], xq[:, 0 : 2 * TILE_P])
        nc.sync.dma_start(ccd_t[:, 256:512], ccd[:, 256:512])
        # rest of x: small chunks first, then 1024-pt chunks, both queues
        CH_EDGES = [2 * TILE_P, 512, 1024]
        while CH_EDGES[-1] < PTS_PER_CORE:
            CH_EDGES.append(min(CH_EDGES[-1] + 1024, PTS_PER_CORE))
        for ci in range(len(CH_EDGES) - 1):
            csl = slice(CH_EDGES[ci], CH_EDGES[ci + 1])
            q = nc.sync if ci % 2 else nc.gpsimd
            q.dma_start(xq_t[:, csl], xq[:, csl])

        # groups: single-tile first and last (shorter head chain and drain),
        # 2-tile batched in between; super-batches of ~32 tiles for the
        # scratch/extraction machinery
        groups = [(0, 1)] + [(1 + 2 * k, 2) for k in range(127)] + [(255, 1)]
        sb_sizes = [31] + [32] * 6 + [33]
        sb_of, acc = [], 0
        for sbi, sz in enumerate(sb_sizes):
            n = 0
            while n < sz:
                n += groups[len(sb_of)][1]
                sb_of.append(sbi)
            acc += sz
        tpos = 0
        gidx = 0
        for gi, (t0g, gr) in enumerate(groups):
            if tpos == 0:
                sbt = sb_sizes[sb_of[gi]]
                sb_base = t0g
                scrbig = scr_pool.tile([128, 33, N_PAIRS], f16)
                accb = idx_pool.tile([128, 17], f32)
                gidx = 0
            # separate psum pools so difs free after the Abs and sums after
            # the scan -- 2 bufs each gives a 3-stage PE->ACT->DVE pipeline
            pss = psum_pool.tile([128, GR, N_PAIRS], f32, tag="sums")
            psd = psum_pool.tile([128, GR, N_PAIRS], f32, tag="difs")
            for j in range(gr):           # difs first: they gate the Abs
                t = t0g + j
                tsl = slice(t * TILE_P, (t + 1) * TILE_P)
                nc.tensor.matmul(
                    psd[:, j, :], xq_t[:, tsl], ccd_t[:],
                    start=True, stop=True,
                )
            for j in range(gr):
                t = t0g + j
                tsl = slice(t * TILE_P, (t + 1) * TILE_P)
                nc.tensor.matmul(
                    pss[:, j, :], xq_t[:, tsl], ccs_t[:],
                    start=True, stop=True,
                )
            absd = abs_pool.tile([128, GR, N_PAIRS], f32)
            nc.scalar.activation(
                absd[:, 0:gr, :], psd[:, 0:gr, :],
                mybir.ActivationFunctionType.Abs,
            )
            nc.vector._custom_dve(
                bop,
                out=scrbig[:, tpos : tpos + gr, :],
                in0=pss[:, 0:gr, :],
                in1=absd[:, 0:gr, :],
                accum_out=accb[:, gidx : gidx + 1],
            )
            tpos += gr
            gidx += 1
            if tpos == sbt:
                # page argmaxes = last element of each page: the idle Pool
                # engine gathers them (SBUF->SBUF is legal for GPSIMD, and
                # this keeps the copies off the saturated DVE), then one
                # contiguous DMA
                idxb = idx_pool.tile([128, 33, 1], f16)
                nc.gpsimd.tensor_copy(
                    idxb[:, 0:sbt, :],
                    scrbig[:, 0:sbt, N_PAIRS - 1 : N_PAIRS],
                )
                nc.sync.dma_start(
                    out[:, sb_base : sb_base + sbt], idxb[:, 0:sbt, :]
                )
                tpos = 0

    nc.compile()
    return nc


def _casc3(A):
    """3-row fp16 cascade summing (exactly, up to fp16 subnormal flush) to A."""
    f16 = np.float16
    n1 = A.astype(f16)
    r1 = A - n1.astype(np.float64)
    n2 = r1.astype(f16)
    n3 = (r1 - n2.astype(np.float64)).astype(f16)
    return n1, n2, n3


def _prep(x: np.ndarray, centers: np.ndarray):
    f16 = np.float16
    xd = x.astype(np.float64)
    cd = centers.astype(np.float64)

    xq = np.empty((67, N_POINTS), f16)
    xq[0:64] = np.ascontiguousarray(xd.T).astype(f16)
    xq[64:67] = f16(1.0)

    cn = (cd * cd).sum(1)
    csum = cd[0::2] + cd[1::2]                  # [512, 64]
    cdif = cd[0::2] - cd[1::2]
    cnsum = (cn[0::2] + cn[1::2]) / 2.0
    cndif = (cn[0::2] - cn[1::2]) / 2.0

    # device computes sum'_g = x.csum - cnsum = (s_2g + s_2g+1)/2
    #             and diff'_g = x.cdif - cndif = (s_2g - s_2g+1)/2
    ccs = np.empty((67, N_PAIRS), f16)
    ccs[0:64] = csum.T.astype(f16)
    ccs[64], ccs[65], ccs[66] = _casc3(-cnsum)
    ccd = np.empty((67, N_PAIRS), f16)
    ccd[0:64] = cdif.T.astype(f16)
    ccd[64], ccd[65], ccd[66] = _casc3(-cndif)
    return xq, ccs, ccd


def kernel(x: np.ndarray, centers: np.ndarray) -> np.ndarray:
    import sys
    if "/opt/trn_rl_repo" not in sys.path:
        sys.path.insert(0, "/opt/trn_rl_repo")
    from concourse.bass_utils import run_bass_kernel_spmd

    x = np.asarray(x, dtype=np.float32)
    centers = np.asarray(centers, dtype=np.float32)

    xq, ccs, ccd = _prep(x, centers)

    if "nc" not in _CACHE:
        _CACHE["nc"] = _build_bass()
    nc = _CACHE["nc"]

    in_maps = []
    for c in range(N_CORES):
        sl = slice(c * PTS_PER_CORE, (c + 1) * PTS_PER_CORE)
        in_maps.append(
            {
                "xq": np.ascontiguousarray(xq[:, sl]),
                "ccs": ccs,
                "ccd": ccd,
            }
        )

    res = run_bass_kernel_spmd(nc, in_maps, list(range(N_CORES)))

    outs = []
    for c in range(N_CORES):
        o = res.results[c]["out"]                 # [128, N_TILES] f16 global idx
        a = np.asarray(o).astype(np.int64)        # page j value = j*512 + pair
        outs.append((a % N_PAIRS).T.reshape(-1))  # point t*128+p -> pair
    g = np.concatenate(outs)                      # winning pair per point

    # within-pair refinement on host: exact fp64 distance compare of the two
    # candidate centers; ties pick the first (matches reference argmin)
    xd = x.astype(np.float64)
    cd = centers.astype(np.float64)
    c0 = cd[2 * g]
    c1 = cd[2 * g + 1]
    d0 = ((xd - c0) ** 2).sum(1)
    d1 = ((xd - c1) ** 2).sum(1)
    ids = np.where(d1 < d0, 2 * g + 1, 2 * g)
    return ids.astype(np.int32)


if __name__ == "__main__":
    rng = np.random.default_rng(0)
    x = rng.normal(size=(N_POINTS, N_FEATURES)).astype(np.float32)
    c = rng.normal(size=(N_CLUSTERS, N_FEATURES)).astype(np.float32)
    ids = kernel(x=x, centers=c)
    d = (
        np.sum(x * x, 1)[:, None]
        - 2.0 * (x @ c.T)
        + np.sum(c * c, 1)[None, :]
    )
    ref = np.argmin(np.abs(d), axis=1)
    print("mismatch:", np.mean(ids != ref))


# revision 41
# speedup vs baseline: 1.0046x; 1.0002x over previous
"""KMeans assignment kernel (retrieval_knn) for 8 Trainium2 NeuronCores.

Computes argmin_k ||x_n - c_k||^2 for x [262144, 64] f32 against centers
[1024, 64] f32, returning int32 cluster ids [262144].

argmin ||x-c||^2 == argmax s, s = 2x.c - ||c||^2.  Centers are pre-combined
on the host into PAIR sums/differences, so the PE emits, per point, the 512
values sum'_g = (s_2g + s_2g+1)/2 and diff'_g = (s_2g - s_2g+1)/2 (fp16
single-pass matmuls).  The ACT engine computes |diff'| (one Abs per 2-tile
group, psum->SBUF), and ONE custom DVE instruction per group scans BOTH
tiles' 512 pairs (argmax over Src0+Src1 = sum'+|diff'| = max(s_2g, s_2g+1)),
amortizing the per-instruction PSUM access latency.  The stock Spec DSL
cannot reset a MAX scan at page boundaries, so `_lower_batched` extends the
lowering with a SUB_DIM_DONE step state that re-seeds the running-max scan
and the accumulator per page, and routes the accumulator's running value to
the out port; the last element of each page is then the page's argmax
(global Idx, so page j yields j*512 + pair), extracted by a tiny strided
DMA straight to DRAM.  The within-pair winner is resolved exactly on the
host (fp64 compare of the two candidate centers, O(N) numpy).

Hardware walls that shape this design (verified against the NEFF verifier):
a DVE instruction may read at most ONE operand from PSUM, GPSIMD cannot
touch PSUM, ACT cannot combine two tensors, matmul output to PSUM is
fp32-only, and custom DVE ops get no 2x perf modes.  Hence every pair costs
one DVE scan cycle; the remaining lever was the per-instruction overhead,
halved here by the 2-tile batching (DVE ~152.5us busy vs 168.4us unbatched).
"""

import numpy as np

N_POINTS = 262144
N_FEATURES = 64
N_CLUSTERS = 1024
N_PAIRS = N_CLUSTERS // 2               # 512
N_CORES = 8
PTS_PER_CORE = N_POINTS // N_CORES      # 32768
TILE_P = 128                            # points per tile (partition dim)
N_TILES = PTS_PER_CORE // TILE_P        # 256
GR = 2                                  # tiles per batched scan / psum group

_CACHE = {}


def _lower_batched(spec, ver):
    """dve_spec.lower() clone with two extensions: (1) a steady/step FSM
    driven by SUB_DIM_DONE where the step state re-seeds every plain MAX
    scan and the accum stage (per-page reset) while consuming the new
    page's first element; (2) out_sel forced to ALU_OUT so the out stream
    carries the accumulator's running value (running argmax)."""
    import concourse.dve_spec as ds
    from concourse.dve_uop import Trigger

    ds._validate_body(spec, ver)
    spec2 = ds._hoist_stream_invariant_ops(spec)
    scans = ds._collect(spec2.body, ds.Scan)
    latches = ds._collect(spec2.body, ds.Latch)
    placement = ds._build_placement(
        spec2, scans, ds.N_STAGES[ver], ds.N_LANES[ver]
    )
    object.__setattr__(placement, "out_sel", ds.OutSel.ALU_OUT)
    states = ds._build_state_machine(spec2, scans, latches, placement)

    step_ov = {}
    for sc in scans:
        if sc.op == ds.AluOp.MAX and sc._subdim_step is None:
            d = placement.node_stage[sc]
            stg = ds._node_as_stage(sc)          # _Stage(MAX, CURR, expr)
            step_ov[d] = ds._Stage(sc.op, ds.MaxNeg, stg.b)
    if placement.accum_stage is not None:
        step_ov[placement.accum_stage] = ds._Stage(
            spec2.accum, ds.MaxNeg, ds.PREV
        )

    body_lvs = ds._body_scan_leaves(spec2)
    consume = (ds.Src0 in body_lvs, ds.Src1 in body_lvs)
    steady_idx = len(states) - 1
    step_idx = steady_idx + 1
    states[steady_idx] = ds._State(
        placement=placement,
        consume=consume,
        trigger=(Trigger.SRC_TENSOR_DONE, Trigger.SUB_DIM_DONE, Trigger.NONE),
        next=(0, step_idx, 0),
    )
    states.append(
        ds._State(
            placement=placement,
            consume=consume,
            overrides=step_ov,
            trigger=(Trigger.SRC_TENSOR_DONE, Trigger.SUB_DIM_DONE, Trigger.COUNT),
            next=(0, step_idx, steady_idx),
            repeat=1,
        )
    )
    out = [ds._assemble(st) for st in states]
    for u in out:
        u.validate(ver)
    return out


def _register_ops():
    """Register the batched custom DVE op (runtime append to dve_ops.OPS).

    BARGMAX_ANT (row 17): subdim op over [P, S, N]; out stream = running
    argmax of (Src0[k] + Src1[k]) with per-page reset; element (s, N-1) of
    the out stream = page s's argmax with global Idx (so page s yields
    s*N + local index).  uops sha self-pinned at registration."""
    import re
    from dataclasses import dataclass

    from concourse import dve_ops
    from concourse.dve_spec import (
        Spec, Src0, Src1, Idx, MaxNeg, AluOp, scan, eq, select, maxx,
    )
    from concourse.dve_uop import DveOpSpec

    if "BARGMAX_ANT" in dve_ops._SUB_OPCODE_FOR_NAME:
        op = next(op for op in dve_ops.OPS if op.name == "BARGMAX_ANT")
        _CACHE["bargmax_op"] = op
        return op

    def _ref(in0, in1, s0, s1, imm2):
        m = in0.astype(np.float32) + in1
        r = np.maximum.accumulate(m, axis=-1)
        base = (
            np.arange(m.shape[-2], dtype=np.float32)[:, None] * m.shape[-1]
            if m.ndim >= 2
            else 0.0
        )
        idx = np.arange(m.shape[-1], dtype=np.float32) + base
        sel = np.where(m == r, idx, -np.finfo(np.float32).max)
        return np.maximum.accumulate(sel, axis=-1)

    @dataclass(frozen=True)
    class BatchedDveOp(dve_ops.DveOp):
        def compile(self, ver):
            key = (self.name, ver)
            if (r := dve_ops._COMPILE_CACHE.get(key)) is not None:
                return r
            from concourse.dve_spec import _has_src1
            result = DveOpSpec(
                name=self.name,
                opcode=dve_ops.get_dve_sub_opcode(self.name),
                uops=_lower_batched(self.spec, ver),
                rd1_en=_has_src1(self.spec),
            )
            got = result.sha(ver)
            if self.uops_sha.get(ver) != got:
                raise ValueError(f"({ver}: {got} drift")
            dve_ops._COMPILE_CACHE[key] = result
            return result

    _mp = Src0 + Src1
    op = BatchedDveOp(
        "BARGMAX_ANT",
        Spec(
            body=select(eq(_mp, scan(AluOp.MAX, _mp)), Idx, MaxNeg),
            accum=maxx,
            reference=_ref,
        ),
        subdim=True,
        uops_sha={},
    )
    dve_ops.OPS.append(op)
    dve_ops._SUB_OPCODE_FOR_NAME[op.name] = (
        dve_ops._CUSTOM_DVE_ROW_BASE + len(dve_ops.OPS) - 1
    )
    dve_ops.CUSTOM_DVE_SPECS[op.name] = op.spec
    for ver in ("v3", "v4"):
        try:
            op.compile(ver)
        except ValueError as e:
            m = re.search(r"\(%s: ([0-9a-f]+) " % ver, str(e))
            if not m:
                raise
            op.uops_sha[ver] = m.group(1)
            op.compile(ver)
    _CACHE["bargmax_op"] = op
    return op


def _build_bass():
    import concourse.bass as bass
    import concourse.bacc as bacc
    import concourse.mybir as mybir
    import concourse.tile as tile
    from contextlib import ExitStack

    bop = _register_ops()

    f16 = mybir.dt.float16
    f32 = mybir.dt.float32

    nc = bacc.Bacc(None, target_bir_lowering=False)

    xq = nc.declare_dram_parameter("xq", [67, PTS_PER_CORE], f16, isOutput=False)
    ccs = nc.declare_dram_parameter("ccs", [67, N_PAIRS], f16, isOutput=False)
    ccd = nc.declare_dram_parameter("ccd", [67, N_PAIRS], f16, isOutput=False)
    out = nc.declare_dram_parameter("out", [128, N_TILES], f16, isOutput=True)

    with tile.TileContext(nc) as tc, ExitStack() as ctx:
        const_pool = ctx.enter_context(tc.tile_pool(name="const", bufs=1))
        psum_pool = ctx.enter_context(
            tc.tile_pool(name="psum", bufs=2, space=bass.MemorySpace.PSUM)
        )
        abs_pool = ctx.enter_context(tc.tile_pool(name="absd", bufs=3))
        scr_pool = ctx.enter_context(tc.tile_pool(name="scr", bufs=2))
        idx_pool = ctx.enter_context(tc.tile_pool(name="idx", bufs=3))
        SB = 16                               # groups per scratch super-batch

        # dummy 1-elem Abs absorbs the 1283ns ACT table load off the critical
        # path; its memset goes FIRST on the gpsimd queue
        xq_t = const_pool.tile([67, PTS_PER_CORE], f16)
        ccs_t = const_pool.tile([67, N_PAIRS], f16)
        ccd_t = const_pool.tile([67, N_PAIRS], f16)
        dummy_in = const_pool.tile([128, 1], f32)
        nc.gpsimd.memset(dummy_in[:], 0)
        dummy_out = const_pool.tile([128, 1], f32)
        nc.scalar.activation(
            dummy_out[:], dummy_in[:], mybir.ActivationFunctionType.Abs
        )
        # a tiny matmul on the memset tile pins pe_busy_start at ~100ns so
        # even the first real matmuls run at the max p-state (the ramp clock
        # never resets on idle)
        warm = psum_pool.tile([128, GR, N_PAIRS], f32, tag="sums")
        nc.tensor.matmul(
            warm[0:1, 0, 0:1], dummy_in[:], dummy_in[:],
            start=True, stop=True,
        )
        # constants split across both cheap queues (dif path gates the head)
        nc.gpsimd.dma_start(ccd_t[:, 0:256], ccd[:, 0:256])
        nc.gpsimd.dma_start(ccs_t[:], ccs[:])
        nc.sync.dma_start(xq_t[:, 0 : 2 * TILE_P], xq[:, 0 : 2 * TILE_P])
        nc.sync.dma_start(ccd_t[:, 256:512], ccd[:, 256:512])
        # rest of x: small chunks first, then 1024-pt chunks, both queues
        CH_EDGES = [2 * TILE_P, 512, 1024]
        while CH_EDGES[-1] < PTS_PER_CORE:
            CH_EDGES.append(min(CH_EDGES[-1] + 1024, PTS_PER_CORE))
        for ci in range(len(CH_EDGES) - 1):
            csl = slice(CH_EDGES[ci], CH_EDGES[ci + 1])
            q = nc.sync if ci % 2 else nc.gpsimd
            q.dma_start(xq_t[:, csl], xq[:, csl])

        # groups: single-tile first and last (shorter head chain and drain),
        # 2-tile batched in between; super-batches of ~32 tiles for the
        # scratch/extraction machinery
        groups = [(0, 1)] + [(1 + 2 * k, 2) for k in range(127)] + [(255, 1)]
        sb_sizes = [31] + [32] * 6 + [28, 5]
        sb_of, acc = [], 0
        for sbi, sz in enumerate(sb_sizes):
            n = 0
            while n < sz:
                n += groups[len(sb_of)][1]
                sb_of.append(sbi)
            acc += sz
        # pre-emit the first two groups' dif matmuls so the second group's
        # Abs isn't delayed behind the first group's sum matmuls (the DVE's
        # only pipeline-fill gap)
        pre_psd = {}
        for pgi in (0, 1):
            pt0, pgr = groups[pgi]
            psd = psum_pool.tile([128, GR, N_PAIRS], f32, tag="difs")
            for j in range(pgr):
                t = pt0 + j
                tsl = slice(t * TILE_P, (t + 1) * TILE_P)
                nc.tensor.matmul(
                    psd[:, j, :], xq_t[:, tsl], ccd_t[:],
                    start=True, stop=True,
                )
            pre_psd[pgi] = psd

        tpos = 0
        gidx = 0
        for gi, (t0g, gr) in enumerate(groups):
            if tpos == 0:
                sbt = sb_sizes[sb_of[gi]]
                sb_base = t0g
                scrbig = scr_pool.tile([128, 33, N_PAIRS], f16)
                accb = idx_pool.tile([128, 17], f32)
                gidx = 0
            # separate psum pools so difs free after the Abs and sums after
            # the scan -- 2 bufs each gives a 3-stage PE->ACT->DVE pipeline
            pss = psum_pool.tile([128, GR, N_PAIRS], f32, tag="sums")
            if gi in pre_psd:
                psd = pre_psd[gi]
            else:
                psd = psum_pool.tile([128, GR, N_PAIRS], f32, tag="difs")
                for j in range(gr):       # difs first: they gate the Abs
                    t = t0g + j
                    tsl = slice(t * TILE_P, (t + 1) * TILE_P)
                    nc.tensor.matmul(
                        psd[:, j, :], xq_t[:, tsl], ccd_t[:],
                        start=True, stop=True,
                    )
            for j in range(gr):
                t = t0g + j
                tsl = slice(t * TILE_P, (t + 1) * TILE_P)
                nc.tensor.matmul(
                    pss[:, j, :], xq_t[:, tsl], ccs_t[:],
                    start=True, stop=True,
                )
            absd = abs_pool.tile([128, GR, N_PAIRS], f32)
            nc.scalar.activation(
                absd[:, 0:gr, :], psd[:, 0:gr, :],
                mybir.ActivationFunctionType.Abs,
            )
            nc.vector._custom_dve(
                bop,
                out=scrbig[:, tpos : tpos + gr, :],
                in0=pss[:, 0:gr, :],
                in1=absd[:, 0:gr, :],
                accum_out=accb[:, gidx : gidx + 1],
            )
            tpos += gr
            gidx += 1
            if tpos == sbt:
                # page argmaxes = last element of each page: the idle Pool
                # engine gathers them (SBUF->SBUF is legal for GPSIMD, and
                # this keeps the copies off the saturated DVE), then one
                # contiguous DMA
                idxb = idx_pool.tile([128, 33, 1], f16)
                nc.gpsimd.tensor_copy(
                    idxb[:, 0:sbt, :],
                    scrbig[:, 0:sbt, N_PAIRS - 1 : N_PAIRS],
                )
                nc.sync.dma_start(
                    out[:, sb_base : sb_base + sbt], idxb[:, 0:sbt, :]
                )
                tpos = 0

    nc.compile()
    return nc


def _casc3(A):
    """3-row fp16 cascade summing (exactly, up to fp16 subnormal flush) to A."""
    f16 = np.float16
    n1 = A.astype(f16)
    r1 = A - n1.astype(np.float64)
    n2 = r1.astype(f16)
    n3 = (r1 - n2.astype(np.float64)).astype(f16)
    return n1, n2, n3


def _prep(x: np.ndarray, centers: np.ndarray):
    f16 = np.float16
    xd = x.astype(np.float64)
    cd = centers.astype(np.float64)

    xq = np.empty((67, N_POINTS), f16)
    xq[0:64] = np.ascontiguousarray(xd.T).astype(f16)
    xq[64:67] = f16(1.0)

    cn = (cd * cd).sum(1)
    csum = cd[0::2] + cd[1::2]                  # [512, 64]
    cdif = cd[0::2] - cd[1::2]
    cnsum = (cn[0::2] + cn[1::2]) / 2.0
    cndif = (cn[0::2] - cn[1::2]) / 2.0

    # device computes sum'_g = x.csum - cnsum = (s_2g + s_2g+1)/2
    #             and diff'_g = x.cdif - cndif = (s_2g - s_2g+1)/2
    ccs = np.empty((67, N_PAIRS), f16)
    ccs[0:64] = csum.T.astype(f16)
    ccs[64], ccs[65], ccs[66] = _casc3(-cnsum)
    ccd = np.empty((67, N_PAIRS), f16)
    ccd[0:64] = cdif.T.astype(f16)
    ccd[64], ccd[65], ccd[66] = _casc3(-cndif)
    return xq, ccs, ccd


def kernel(x: np.ndarray, centers: np.ndarray) -> np.ndarray:
    import sys
    if "/opt/trn_rl_repo" not in sys.path:
        sys.path.insert(0, "/opt/trn_rl_repo")
    from concourse.bass_utils import run_bass_kernel_spmd

    x = np.asarray(x, dtype=np.float32)
    centers = np.asarray(centers, dtype=np.float32)

    xq, ccs, ccd = _prep(x, centers)

    if "nc" not in _CACHE:
        _CACHE["nc"] = _build_bass()
    nc = _CACHE["nc"]

    in_maps = []
    for c in range(N_CORES):
        sl = slice(c * PTS_PER_CORE, (c + 1) * PTS_PER_CORE)
        in_maps.append(
            {
                "xq": np.ascontiguousarray(xq[:, sl]),
                "ccs": ccs,
                "ccd": ccd,
            }
        )

    res = run_bass_kernel_spmd(nc, in_maps, list(range(N_CORES)))

    outs = []
    for c in range(N_CORES):
        o = res.results[c]["out"]                 # [128, N_TILES] f16 global idx
        a = np.asarray(o).astype(np.int64)        # page j value = j*512 + pair
        outs.append((a % N_PAIRS).T.reshape(-1))  # point t*128+p -> pair
    g = np.concatenate(outs)                      # winning pair per point

    # within-pair refinement on host: exact fp64 distance compare of the two
    # candidate centers; ties pick the first (matches reference argmin)
    xd = x.astype(np.float64)
    cd = centers.astype(np.float64)
    c0 = cd[2 * g]
    c1 = cd[2 * g + 1]
    d0 = ((xd - c0) ** 2).sum(1)
    d1 = ((xd - c1) ** 2).sum(1)
    ids = np.where(d1 < d0, 2 * g + 1, 2 * g)
    return ids.astype(np.int32)


if __name__ == "__main__":
    rng = np.random.default_rng(0)
    x = rng.normal(size=(N_POINTS, N_FEATURES)).astype(np.float32)
    c = rng.normal(size=(N_CLUSTERS, N_FEATURES)).astype(np.float32)
    ids = kernel(x=x, centers=c)
    d = (
        np.sum(x * x, 1)[:, None]
        - 2.0 * (x @ c.T)
        + np.sum(c * c, 1)[None, :]
    )
    ref = np.argmin(np.abs(d), axis=1)
    print("mismatch:", np.mean(ids != ref))


# revision 42
# speedup vs baseline: 1.0053x; 1.0007x over previous
"""KMeans assignment kernel (retrieval_knn) for 8 Trainium2 NeuronCores.

Computes argmin_k ||x_n - c_k||^2 for x [262144, 64] f32 against centers
[1024, 64] f32, returning int32 cluster ids [262144].

argmin ||x-c||^2 == argmax s, s = 2x.c - ||c||^2.  Centers are pre-combined
on the host into PAIR sums/differences, so the PE emits, per point, the 512
values sum'_g = (s_2g + s_2g+1)/2 and diff'_g = (s_2g - s_2g+1)/2 (fp16
single-pass matmuls).  The ACT engine computes |diff'| (one Abs per 2-tile
group, psum->SBUF), and ONE custom DVE instruction per group scans BOTH
tiles' 512 pairs (argmax over Src0+Src1 = sum'+|diff'| = max(s_2g, s_2g+1)),
amortizing the per-instruction PSUM access latency.  The stock Spec DSL
cannot reset a MAX scan at page boundaries, so `_lower_batched` extends the
lowering with a SUB_DIM_DONE step state that re-seeds the running-max scan
and the accumulator per page, and routes the accumulator's running value to
the out port; the last element of each page is then the page's argmax
(global Idx, so page j yields j*512 + pair), extracted by a tiny strided
DMA straight to DRAM.  The within-pair winner is resolved exactly on the
host (fp64 compare of the two candidate centers, O(N) numpy).

Hardware walls that shape this design (verified against the NEFF verifier):
a DVE instruction may read at most ONE operand from PSUM, GPSIMD cannot
touch PSUM, ACT cannot combine two tensors, matmul output to PSUM is
fp32-only, and custom DVE ops get no 2x perf modes.  Hence every pair costs
one DVE scan cycle; the remaining lever was the per-instruction overhead,
halved here by the 2-tile batching (DVE ~152.5us busy vs 168.4us unbatched).
"""

import numpy as np

N_POINTS = 262144
N_FEATURES = 64
N_CLUSTERS = 1024
N_PAIRS = N_CLUSTERS // 2               # 512
N_CORES = 8
PTS_PER_CORE = N_POINTS // N_CORES      # 32768
TILE_P = 128                            # points per tile (partition dim)
N_TILES = PTS_PER_CORE // TILE_P        # 256
GR = 2                                  # tiles per batched scan / psum group

_CACHE = {}


def _lower_batched(spec, ver):
    """dve_spec.lower() clone with two extensions: (1) a steady/step FSM
    driven by SUB_DIM_DONE where the step state re-seeds every plain MAX
    scan and the accum stage (per-page reset) while consuming the new
    page's first element; (2) out_sel forced to ALU_OUT so the out stream
    carries the accumulator's running value (running argmax)."""
    import concourse.dve_spec as ds
    from concourse.dve_uop import Trigger

    ds._validate_body(spec, ver)
    spec2 = ds._hoist_stream_invariant_ops(spec)
    scans = ds._collect(spec2.body, ds.Scan)
    latches = ds._collect(spec2.body, ds.Latch)
    placement = ds._build_placement(
        spec2, scans, ds.N_STAGES[ver], ds.N_LANES[ver]
    )
    object.__setattr__(placement, "out_sel", ds.OutSel.ALU_OUT)
    states = ds._build_state_machine(spec2, scans, latches, placement)

    step_ov = {}
    for sc in scans:
        if sc.op == ds.AluOp.MAX and sc._subdim_step is None:
            d = placement.node_stage[sc]
            stg = ds._node_as_stage(sc)          # _Stage(MAX, CURR, expr)
            step_ov[d] = ds._Stage(sc.op, ds.MaxNeg, stg.b)
    if placement.accum_stage is not None:
        step_ov[placement.accum_stage] = ds._Stage(
            spec2.accum, ds.MaxNeg, ds.PREV
        )

    body_lvs = ds._body_scan_leaves(spec2)
    consume = (ds.Src0 in body_lvs, ds.Src1 in body_lvs)
    steady_idx = len(states) - 1
    step_idx = steady_idx + 1
    states[steady_idx] = ds._State(
        placement=placement,
        consume=consume,
        trigger=(Trigger.SRC_TENSOR_DONE, Trigger.SUB_DIM_DONE, Trigger.NONE),
        next=(0, step_idx, 0),
    )
    states.append(
        ds._State(
            placement=placement,
            consume=consume,
            overrides=step_ov,
            trigger=(Trigger.SRC_TENSOR_DONE, Trigger.SUB_DIM_DONE, Trigger.COUNT),
            next=(0, step_idx, steady_idx),
            repeat=1,
        )
    )
    out = [ds._assemble(st) for st in states]
    for u in out:
        u.validate(ver)
    return out


def _register_ops():
    """Register the batched custom DVE op (runtime append to dve_ops.OPS).

    BARGMAX_ANT (row 17): subdim op over [P, S, N]; out stream = running
    argmax of (Src0[k] + Src1[k]) with per-page reset; element (s, N-1) of
    the out stream = page s's argmax with global Idx (so page s yields
    s*N + local index).  uops sha self-pinned at registration."""
    import re
    from dataclasses import dataclass

    from concourse import dve_ops
    from concourse.dve_spec import (
        Spec, Src0, Src1, Idx, MaxNeg, AluOp, scan, eq, select, maxx,
    )
    from concourse.dve_uop import DveOpSpec

    if "BARGMAX_ANT" in dve_ops._SUB_OPCODE_FOR_NAME:
        op = next(op for op in dve_ops.OPS if op.name == "BARGMAX_ANT")
        _CACHE["bargmax_op"] = op
        return op

    def _ref(in0, in1, s0, s1, imm2):
        m = in0.astype(np.float32) + in1
        r = np.maximum.accumulate(m, axis=-1)
        base = (
            np.arange(m.shape[-2], dtype=np.float32)[:, None] * m.shape[-1]
            if m.ndim >= 2
            else 0.0
        )
        idx = np.arange(m.shape[-1], dtype=np.float32) + base
        sel = np.where(m == r, idx, -np.finfo(np.float32).max)
        return np.maximum.accumulate(sel, axis=-1)

    @dataclass(frozen=True)
    class BatchedDveOp(dve_ops.DveOp):
        def compile(self, ver):
            key = (self.name, ver)
            if (r := dve_ops._COMPILE_CACHE.get(key)) is not None:
                return r
            from concourse.dve_spec import _has_src1
            result = DveOpSpec(
                name=self.name,
                opcode=dve_ops.get_dve_sub_opcode(self.name),
                uops=_lower_batched(self.spec, ver),
                rd1_en=_has_src1(self.spec),
            )
            got = result.sha(ver)
            if self.uops_sha.get(ver) != got:
                raise ValueError(f"({ver}: {got} drift")
            dve_ops._COMPILE_CACHE[key] = result
            return result

    _mp = Src0 + Src1
    op = BatchedDveOp(
        "BARGMAX_ANT",
        Spec(
            body=select(eq(_mp, scan(AluOp.MAX, _mp)), Idx, MaxNeg),
            accum=maxx,
            reference=_ref,
        ),
        subdim=True,
        uops_sha={},
    )
    dve_ops.OPS.append(op)
    dve_ops._SUB_OPCODE_FOR_NAME[op.name] = (
        dve_ops._CUSTOM_DVE_ROW_BASE + len(dve_ops.OPS) - 1
    )
    dve_ops.CUSTOM_DVE_SPECS[op.name] = op.spec
    for ver in ("v3", "v4"):
        try:
            op.compile(ver)
        except ValueError as e:
            m = re.search(r"\(%s: ([0-9a-f]+) " % ver, str(e))
            if not m:
                raise
            op.uops_sha[ver] = m.group(1)
            op.compile(ver)
    _CACHE["bargmax_op"] = op
    return op


def _build_bass():
    import concourse.bass as bass
    import concourse.bacc as bacc
    import concourse.mybir as mybir
    import concourse.tile as tile
    from contextlib import ExitStack

    bop = _register_ops()

    f16 = mybir.dt.float16
    f32 = mybir.dt.float32

    nc = bacc.Bacc(None, target_bir_lowering=False)

    xq = nc.declare_dram_parameter("xq", [67, PTS_PER_CORE], f16, isOutput=False)
    ccs = nc.declare_dram_parameter("ccs", [67, N_PAIRS], f16, isOutput=False)
    ccd = nc.declare_dram_parameter("ccd", [67, N_PAIRS], f16, isOutput=False)
    out = nc.declare_dram_parameter("out", [128, N_TILES], f16, isOutput=True)

    with tile.TileContext(nc) as tc, ExitStack() as ctx:
        const_pool = ctx.enter_context(tc.tile_pool(name="const", bufs=1))
        psum_pool = ctx.enter_context(
            tc.tile_pool(name="psum", bufs=2, space=bass.MemorySpace.PSUM)
        )
        abs_pool = ctx.enter_context(tc.tile_pool(name="absd", bufs=3))
        scr_pool = ctx.enter_context(tc.tile_pool(name="scr", bufs=2))
        idx_pool = ctx.enter_context(tc.tile_pool(name="idx", bufs=3))
        SB = 16                               # groups per scratch super-batch

        # dummy 1-elem Abs absorbs the 1283ns ACT table load off the critical
        # path; its memset goes FIRST on the gpsimd queue
        xq_t = const_pool.tile([67, PTS_PER_CORE], f16)
        ccs_t = const_pool.tile([67, N_PAIRS], f16)
        ccd_t = const_pool.tile([67, N_PAIRS], f16)
        dummy_in = const_pool.tile([128, 1], f32)
        nc.gpsimd.memset(dummy_in[:], 0)
        dummy_out = const_pool.tile([128, 1], f32)
        nc.scalar.activation(
            dummy_out[:], dummy_in[:], mybir.ActivationFunctionType.Abs
        )
        # a tiny matmul on the memset tile pins pe_busy_start at ~100ns so
        # even the first real matmuls run at the max p-state (the ramp clock
        # never resets on idle)
        warm = psum_pool.tile([128, GR, N_PAIRS], f32, tag="sums")
        nc.tensor.matmul(
            warm[0:1, 0, 0:1], dummy_in[:], dummy_in[:],
            start=True, stop=True,
        )
        # constants split across both cheap queues (dif path gates the head)
        nc.gpsimd.dma_start(ccd_t[:, 0:256], ccd[:, 0:256])
        nc.gpsimd.dma_start(ccs_t[:], ccs[:])
        nc.sync.dma_start(xq_t[:, 0 : 2 * TILE_P], xq[:, 0 : 2 * TILE_P])
        nc.sync.dma_start(ccd_t[:, 256:512], ccd[:, 256:512])
        # rest of x: small chunks first, then 1024-pt chunks, both queues
        CH_EDGES = [2 * TILE_P, 512, 1024]
        while CH_EDGES[-1] < PTS_PER_CORE:
            CH_EDGES.append(min(CH_EDGES[-1] + 1024, PTS_PER_CORE))
        for ci in range(len(CH_EDGES) - 1):
            csl = slice(CH_EDGES[ci], CH_EDGES[ci + 1])
            q = nc.sync if ci % 2 else nc.gpsimd
            q.dma_start(xq_t[:, csl], xq[:, csl])

        # groups: single-tile first and last (shorter head chain and drain),
        # 2-tile batched in between; super-batches of ~32 tiles for the
        # scratch/extraction machinery
        groups = [(0, 1)] + [(1 + 2 * k, 2) for k in range(127)] + [(255, 1)]
        sb_sizes = [31] + [32] * 6 + [28, 5]
        sb_of, acc = [], 0
        for sbi, sz in enumerate(sb_sizes):
            n = 0
            while n < sz:
                n += groups[len(sb_of)][1]
                sb_of.append(sbi)
            acc += sz
        # pre-emit the first two groups' dif matmuls so the second group's
        # Abs isn't delayed behind the first group's sum matmuls (the DVE's
        # only pipeline-fill gap)
        pre_psd = {}
        for pgi in (0, 1):
            pt0, pgr = groups[pgi]
            psd = psum_pool.tile([128, GR, N_PAIRS], f32, tag="difs")
            for j in range(pgr):
                t = pt0 + j
                tsl = slice(t * TILE_P, (t + 1) * TILE_P)
                nc.tensor.matmul(
                    psd[:, j, :], xq_t[:, tsl], ccd_t[:],
                    start=True, stop=True,
                )
            pre_psd[pgi] = psd

        tpos = 0
        gidx = 0
        for gi, (t0g, gr) in enumerate(groups):
            if tpos == 0:
                sbt = sb_sizes[sb_of[gi]]
                sb_base = t0g
                scrbig = scr_pool.tile([128, 33, N_PAIRS], f16)
                accb = idx_pool.tile([128, 17], f32)
                gidx = 0
            # separate psum pools so difs free after the Abs and sums after
            # the scan -- 2 bufs each gives a 3-stage PE->ACT->DVE pipeline
            pss = psum_pool.tile([128, GR, N_PAIRS], f32, tag="sums")
            if gi in pre_psd:
                psd = pre_psd[gi]
            else:
                psd = psum_pool.tile([128, GR, N_PAIRS], f32, tag="difs")
                for j in range(gr):       # difs first: they gate the Abs
                    t = t0g + j
                    tsl = slice(t * TILE_P, (t + 1) * TILE_P)
                    nc.tensor.matmul(
                        psd[:, j, :], xq_t[:, tsl], ccd_t[:],
                        start=True, stop=True,
                    )
            for j in range(gr):
                t = t0g + j
                tsl = slice(t * TILE_P, (t + 1) * TILE_P)
                nc.tensor.matmul(
                    pss[:, j, :], xq_t[:, tsl], ccs_t[:],
                    start=True, stop=True,
                )
            absd = abs_pool.tile([128, GR, N_PAIRS], f32)
            nc.scalar.activation(
                absd[:, 0:gr, :], psd[:, 0:gr, :],
                mybir.ActivationFunctionType.Abs,
            )
            nc.vector._custom_dve(
                bop,
                out=scrbig[:, tpos : tpos + gr, :],
                in0=pss[:, 0:gr, :],
                in1=absd[:, 0:gr, :],
                accum_out=accb[:, gidx : gidx + 1],
            )
            tpos += gr
            gidx += 1
            if tpos == sbt:
                # page argmaxes = last element of each page: one strided DMA
                # straight to DRAM (descriptor cost hides in the 2-buf
                # scratch slack; no copy hop on the tail-critical path)
                nc.sync.dma_start(
                    out[:, sb_base : sb_base + sbt],
                    scrbig[:, 0:sbt, N_PAIRS - 1 : N_PAIRS],
                )
                tpos = 0

    nc.compile()
    return nc


def _casc3(A):
    """3-row fp16 cascade summing (exactly, up to fp16 subnormal flush) to A."""
    f16 = np.float16
    n1 = A.astype(f16)
    r1 = A - n1.astype(np.float64)
    n2 = r1.astype(f16)
    n3 = (r1 - n2.astype(np.float64)).astype(f16)
    return n1, n2, n3


def _prep(x: np.ndarray, centers: np.ndarray):
    f16 = np.float16
    xd = x.astype(np.float64)
    cd = centers.astype(np.float64)

    xq = np.empty((67, N_POINTS), f16)
    xq[0:64] = np.ascontiguousarray(xd.T).astype(f16)
    xq[64:67] = f16(1.0)

    cn = (cd * cd).sum(1)
    csum = cd[0::2] + cd[1::2]                  # [512, 64]
    cdif = cd[0::2] - cd[1::2]
    cnsum = (cn[0::2] + cn[1::2]) / 2.0
    cndif = (cn[0::2] - cn[1::2]) / 2.0

    # device computes sum'_g = x.csum - cnsum = (s_2g + s_2g+1)/2
    #             and diff'_g = x.cdif - cndif = (s_2g - s_2g+1)/2
    ccs = np.empty((67, N_PAIRS), f16)
    ccs[0:64] = csum.T.astype(f16)
    ccs[64], ccs[65], ccs[66] = _casc3(-cnsum)
    ccd = np.empty((67, N_PAIRS), f16)
    ccd[0:64] = cdif.T.astype(f16)
    ccd[64], ccd[65], ccd[66] = _casc3(-cndif)
    return xq, ccs, ccd


def kernel(x: np.ndarray, centers: np.ndarray) -> np.ndarray:
    import sys
    if "/opt/trn_rl_repo" not in sys.path:
        sys.path.insert(0, "/opt/trn_rl_repo")
    from concourse.bass_utils import run_bass_kernel_spmd

    x = np.asarray(x, dtype=np.float32)
    centers = np.asarray(centers, dtype=np.float32)

    xq, ccs, ccd = _prep(x, centers)

    if "nc" not in _CACHE:
        _CACHE["nc"] = _build_bass()
    nc = _CACHE["nc"]

    in_maps = []
    for c in range(N_CORES):
        sl = slice(c * PTS_PER_CORE, (c + 1) * PTS_PER_CORE)
        in_maps.append(
            {
                "xq": np.ascontiguousarray(xq[:, sl]),
                "ccs": ccs,
                "ccd": ccd,
            }
        )

    res = run_bass_kernel_spmd(nc, in_maps, list(range(N_CORES)))

    outs = []
    for c in range(N_CORES):
        o = res.results[c]["out"]                 # [128, N_TILES] f16 global idx
        a = np.asarray(o).astype(np.int64)        # page j value = j*512 + pair
        outs.append((a % N_PAIRS).T.reshape(-1))  # point t*128+p -> pair
    g = np.concatenate(outs)                      # winning pair per point

    # within-pair refinement on host: exact fp64 distance compare of the two
    # candidate centers; ties pick the first (matches reference argmin)
    xd = x.astype(np.float64)
    cd = centers.astype(np.float64)
    c0 = cd[2 * g]
    c1 = cd[2 * g + 1]
    d0 = ((xd - c0) ** 2).sum(1)
    d1 = ((xd - c1) ** 2).sum(1)
    ids = np.where(d1 < d0, 2 * g + 1, 2 * g)
    return ids.astype(np.int32)


if __name__ == "__main__":
    rng = np.random.default_rng(0)
    x = rng.normal(size=(N_POINTS, N_FEATURES)).astype(np.float32)
    c = rng.normal(size=(N_CLUSTERS, N_FEATURES)).astype(np.float32)
    ids = kernel(x=x, centers=c)
    d = (
        np.sum(x * x, 1)[:, None]
        - 2.0 * (x @ c.T)
        + np.sum(c * c, 1)[None, :]
    )
    ref = np.argmin(np.abs(d), axis=1)
    print("mismatch:", np.mean(ids != ref))
